# revision 2
# baseline (speedup 1.0000x reference)
"""Bahdanau attention Trainium2 kernel (Bass/Tile), data-parallel over 8 NeuronCores.

Reference computation (per batch b):
  q_proj = query @ Wa_w.T + Wa_b                      [B, H]
  k_proj = keys @ Ua_w.T + Ua_b                       [B, S, H]
  e      = tanh(q_proj[:, None, :] + k_proj)          [B, S, H]
  scores = e @ Va_w[0] + Va_b                         [B, S]   (Va_b drops out of softmax)
  weights = softmax(scores, -1)                       [B, S]
  context = weights @ keys                            [B, H]

Sharding: batch dim across 8 cores (4 each), small weights replicated.
On-chip layout: h must sit on SBUF partitions to contract over it, so keys
tiles are cast to bf16 (DVE) and transposed via the DMA xbar (16x128 tiles).
The big matmul runs in bf16 with fp32 PSUM accumulation.
"""
import numpy as np
import ml_dtypes

from concourse import bacc
import concourse.mybir as mybir
import concourse.tile as tile
from concourse.bass_utils import run_bass_kernel_spmd

F32 = mybir.dt.float32
BF16 = mybir.dt.bfloat16
AF = mybir.ActivationFunctionType
ALU = mybir.AluOpType

B, S, H = 32, 2048, 1024
NCORES = 8
BLOC = B // NCORES          # 4 batches per core
P = 128
HC = H // P                 # 8 h-chunks
OC = H // P                 # 8 o-chunks
ST = S // 512               # 4 s-tiles for the big matmul
SC = S // P                 # 16 s-chunks of 128

_NC = None


def build_kernel():
    nc = bacc.Bacc("TRN2")
    keys = nc.dram_tensor("keys", [BLOC, S, H], F32, kind="ExternalInput")
    q_t = nc.dram_tensor("q_t", [P, HC, BLOC], F32, kind="ExternalInput")
    ua_t = nc.dram_tensor("ua_t", [P, HC, H], F32, kind="ExternalInput")
    wa_t = nc.dram_tensor("wa_t", [P, HC, H], F32, kind="ExternalInput")
    va_t = nc.dram_tensor("va_t", [P, OC], F32, kind="ExternalInput")
    uab_t = nc.dram_tensor("uab_t", [P, OC], F32, kind="ExternalInput")
    wab_t = nc.dram_tensor("wab_t", [P, OC], F32, kind="ExternalInput")
    out_ctx = nc.dram_tensor("out_ctx", [BLOC, H], F32, kind="ExternalOutput")
    out_w = nc.dram_tensor("out_w", [BLOC, S], F32, kind="ExternalOutput")

    with tile.TileContext(nc) as tc:
        with tc.tile_pool(name="persist", bufs=1) as persist:
            ua_sb = persist.tile([P, HC, H], BF16)       # Ua^T, bf16
            va_sb = persist.tile([P, OC], BF16)          # Va columns
            qb = persist.tile([P, OC, BLOC], F32)        # q_proj + Wa_b + Ua_b
            w_row = persist.tile([16, S], BF16)          # softmax weights staging row

            # ---- init: load/cast small weights, compute q_proj bias ----
            with (
                tc.tile_pool(name="init", bufs=2) as initp,
                tc.tile_pool(name="qps", bufs=2, space="PSUM") as qps,
            ):
                wa_sb = initp.tile([P, HC, H], BF16, tag="wa")
                for j in range(4):
                    st32 = initp.tile([P, 2, H], F32, tag="ist")
                    nc.sync.dma_start(out=st32[:], in_=ua_t[:, 2 * j:2 * j + 2, :])
                    nc.vector.tensor_copy(out=ua_sb[:, 2 * j:2 * j + 2, :], in_=st32[:])
                for j in range(4):
                    st32 = initp.tile([P, 2, H], F32, tag="ist")
                    nc.sync.dma_start(out=st32[:], in_=wa_t[:, 2 * j:2 * j + 2, :])
                    nc.vector.tensor_copy(out=wa_sb[:, 2 * j:2 * j + 2, :], in_=st32[:])
                q32 = initp.tile([P, HC, BLOC], F32, tag="q32")
                nc.sync.dma_start(out=q32[:], in_=q_t[:])
                q_sb = initp.tile([P, HC, BLOC], BF16, tag="qbf")
                nc.vector.tensor_copy(out=q_sb[:], in_=q32[:])
                va32 = initp.tile([P, OC], F32, tag="va32")
                nc.sync.dma_start(out=va32[:], in_=va_t[:])
                nc.vector.tensor_copy(out=va_sb[:], in_=va32[:])
                uab = initp.tile([P, OC], F32, tag="uab")
                nc.sync.dma_start(out=uab[:], in_=uab_t[:])
                wab = initp.tile([P, OC], F32, tag="wab")
                nc.sync.dma_start(out=wab[:], in_=wab_t[:])

                for oc in range(OC):
                    psq = qps.tile([P, BLOC], F32, tag="q")
                    for hc in range(HC):
                        nc.tensor.matmul(
                            psq[:], wa_sb[:, hc, oc * P:(oc + 1) * P],
                            q_sb[:, hc, :], start=(hc == 0), stop=(hc == HC - 1),
                        )
                    nc.vector.tensor_scalar(
                        out=qb[:, oc, :], in0=psq[:],
                        scalar1=wab[:, oc:oc + 1], scalar2=uab[:, oc:oc + 1],
                        op0=ALU.add, op1=ALU.add,
                    )
                nc.vector.memset(w_row[:], 0.0)

            # ---- main pools ----
            with (
                tc.tile_pool(name="stage", bufs=2) as stagep,
                tc.tile_pool(name="nat", bufs=2) as natp,
                tc.tile_pool(name="kt", bufs=1) as ktp,
                tc.tile_pool(name="ep", bufs=1) as ep,
                tc.tile_pool(name="vec", bufs=1) as vecp,
                tc.tile_pool(name="mmp", bufs=6, space="PSUM") as mmp,
                tc.tile_pool(name="vps", bufs=2, space="PSUM") as vps,
            ):
                for b in range(BLOC):
                    # load + cast keys[b] -> natural bf16 [p, sc, h], s = sc*128 + p
                    natb = natp.tile([P, SC, H], BF16, tag="nat")
                    ksrc = keys[b].rearrange("(sc p) h -> p sc h", p=P)
                    for j in range(4):
                        st32 = stagep.tile([P, 4, H], F32, tag="st")
                        nc.sync.dma_start(out=st32[:], in_=ksrc[:, 4 * j:4 * j + 4, :])
                        nc.vector.tensor_copy(out=natb[:, 4 * j:4 * j + 4, :], in_=st32[:])

                    # xbar transpose -> keys^T bf16 [p, hc, s], h = hc*128 + p
                    kt = ktp.tile([P, HC, S], BF16, tag="kt")
                    for sc in range(SC):
                        nc.sync.dma_start(
                            out=kt[:, :, sc * P:(sc + 1) * P],
                            in_=natb[:, sc, :],
                            transpose=True,
                        )

                    # e^T = tanh(Ua keys^T + qb)  [o on partitions, s free], bf16
                    e_sb = ep.tile([P, OC, S], BF16, tag="e")
                    for oc in range(OC):
                        pss = [mmp.tile([P, 512], F32, tag="mm", name=f"mm{st}") for st in range(ST)]
                        for hc in range(HC):
                            for st in range(ST):
                                nc.tensor.matmul(
                                    pss[st][:], ua_sb[:, hc, oc * P:(oc + 1) * P],
                                    kt[:, hc, st * 512:(st + 1) * 512],
                                    start=(hc == 0), stop=(hc == HC - 1),
                                )
                        for st in range(ST):
                            nc.scalar.activation(
                                out=e_sb[:, oc, st * 512:(st + 1) * 512], in_=pss[st][:],
                                func=AF.Tanh, bias=qb[:, oc, b:b + 1], scale=1.0,
                            )

                    # scores = Va . e^T  (contract o on partitions via PE)
                    sc_sb = vecp.tile([1, S], F32, tag="scores")
                    for st in range(ST):
                        pscore = vps.tile([1, 512], F32, tag="vec")
                        for oc in range(OC):
                            nc.tensor.matmul(
                                pscore[:], va_sb[:, oc:oc + 1],
                                e_sb[:, oc, st * 512:(st + 1) * 512],
                                start=(oc == 0), stop=(oc == OC - 1),
                            )
                        nc.scalar.copy(out=sc_sb[:, st * 512:(st + 1) * 512], in_=pscore[:])

                    # softmax over s (single partition, fp32)
                    mx = vecp.tile([1, 1], F32, tag="mx")
                    nc.vector.reduce_max(out=mx[:], in_=sc_sb[:], axis=mybir.AxisListType.X)
                    nmx = vecp.tile([1, 1], F32, tag="nmx")
                    nc.vector.tensor_scalar_mul(out=nmx[:], in0=mx[:], scalar1=-1.0)
                    wexp = vecp.tile([1, S], F32, tag="wexp")
                    ssum = vecp.tile([1, 1], F32, tag="ssum")
                    nc.scalar.activation(
                        out=wexp[:], in_=sc_sb[:], func=AF.Exp,
                        bias=nmx[:], scale=1.0, accum_out=ssum[:],
                    )
                    rs = vecp.tile([1, 1], F32, tag="rs")
                    nc.vector.reciprocal(out=rs[:], in_=ssum[:])
                    # bf16 weights (for the context matmul)
                    nc.vector.tensor_scalar_mul(out=w_row[0:1, :], in0=wexp[:], scalar1=rs[:])
                    # fp32 weights output (in place), then store
                    nc.vector.tensor_scalar_mul(out=wexp[:], in0=wexp[:], scalar1=rs[:])
                    nc.sync.dma_start(out=out_w[b:b + 1, :], in_=wexp[:])

                    # transpose weights row -> [p, sc, 16] columns (s = sc*128 + p)
                    wt = vecp.tile([P, SC, 16], BF16, tag="wt")
                    nc.sync.dma_start(out=wt[:], in_=w_row[:], transpose=True)

                    # context = weights @ keys (contract s on partitions)
                    ctx_sb = vecp.tile([1, H], F32, tag="ctx")
                    for hh in range(2):
                        pctx = vps.tile([1, 512], F32, tag="vec")
                        for sc in range(SC):
                            nc.tensor.matmul(
                                pctx[:], wt[:, sc, 0:1],
                                natb[:, sc, hh * 512:(hh + 1) * 512],
                                start=(sc == 0), stop=(sc == SC - 1),
                            )
                        nc.scalar.copy(out=ctx_sb[:, hh * 512:(hh + 1) * 512], in_=pctx[:])
                    nc.sync.dma_start(out=out_ctx[b:b + 1, :], in_=ctx_sb[:])

    nc.finalize()
    return nc


def _get_nc():
    global _NC
    if _NC is None:
        _NC = build_kernel()
    return _NC


def _prep_core_inputs(inputs):
    keys = np.asarray(inputs["keys"], dtype=np.float32)
    query = np.asarray(inputs["query"], dtype=np.float32)
    ua_w = np.asarray(inputs["Ua_w"], dtype=np.float32)
    wa_w = np.asarray(inputs["Wa_w"], dtype=np.float32)
    va_w = np.asarray(inputs["Va_w"], dtype=np.float32)
    ua_b = np.asarray(inputs["Ua_b"], dtype=np.float32)
    wa_b = np.asarray(inputs["Wa_b"], dtype=np.float32)

    ua_t = np.ascontiguousarray(ua_w.T.reshape(HC, P, H).swapaxes(0, 1))
    wa_t = np.ascontiguousarray(wa_w.T.reshape(HC, P, H).swapaxes(0, 1))
    va_t = np.ascontiguousarray(va_w[0].reshape(OC, P).T)
    uab_t = np.ascontiguousarray(ua_b.reshape(OC, P).T)
    wab_t = np.ascontiguousarray(wa_b.reshape(OC, P).T)

    in_maps = []
    for c in range(NCORES):
        b0 = c * BLOC
        q_shard = query[b0:b0 + BLOC]                     # [BLOC, H]
        q_t = np.ascontiguousarray(q_shard.T.reshape(HC, P, BLOC).swapaxes(0, 1))
        in_maps.append({
            "keys": np.ascontiguousarray(keys[b0:b0 + BLOC]),
            "q_t": q_t,
            "ua_t": ua_t,
            "wa_t": wa_t,
            "va_t": va_t,
            "uab_t": uab_t,
            "wab_t": wab_t,
        })
    return in_maps


def run_sharded(inputs, **spmd_kwargs):
    nc = _get_nc()
    in_maps = _prep_core_inputs(inputs)
    res = run_bass_kernel_spmd(nc, in_maps, core_ids=list(range(NCORES)), **spmd_kwargs)
    ctx = np.concatenate([r["out_ctx"] for r in res.results], axis=0)
    wgt = np.concatenate([r["out_w"] for r in res.results], axis=0)
    return (ctx, wgt), res


def kernel(**inputs):
    (ctx, wgt), _ = run_sharded(inputs)
    return (ctx, wgt)
